# revision 1
# baseline (speedup 1.0000x reference)
"""EquiRNN Trainium2 kernel.

Math (reference): bidirectional group-equivariant tanh-RNN.
  emb[t,g]   = W_e[perms[g, tok_t]]            (sequence is one-hot, perms are cyclic shifts)
  xproj      = emb @ Wx + b
  fwd:  h_t  = tanh(xproj[t] + h_{t-1} @ Wh)
  bwd:  same on rev = [xproj[T-2], ..., xproj[0], xproj[T-1]]
  out hidden_all (T, G, 2K), ht (G, 2K) = hidden_all[-1]

Strategy:
  - Host folds weights once: W_proj_g = roll(W_e @ Wx, -g*(V//G), axis=0) + b  (input-independent).
  - G=24 group copies sharded 3 per core across 8 cores; each core runs fwd+bwd
    for its 3 groups (6 independent chains, shared Wh).
  - Device phase A: transpose `sequence` on the PE, then xprojT_g = W_proj_g^T @ seq^T
    (exact for arbitrary sequence), staged to DRAM in the recurrence's transposed
    layout (128, kc, g, T).
  - Device phase B: serial recurrence, state kept transposed (k on partitions):
    preT[:, j, chains] = sum_c Wh[c-chunk, j-chunk]^T @ hT[c-chunk, chains]  (16 matmuls)
    then DVE add of xT and ACT tanh back into the rolling state/output buffer.
  - All fp32: the recurrence is mildly chaotic (error growth ~1.16x/step), so low
    precision anywhere in the loop is amplified ~100x into the outputs.
  - Host un-transposes outputs, un-reverses the bwd stream, concatenates cores.
"""

import numpy as np

G, T, V, K = 24, 2048, 2048, 512
NCORES = 8
GL = G // NCORES          # groups per core = 3
NCH = 2 * GL              # chains per core (fwd 0..GL-1, bwd GL..2GL-1)
KC = K // 128             # k chunks = 4
SHIFT = V // G            # 85
TB = 256                  # recurrence block size
TGB = 512                 # phase-A GEMM t-block


def build_program(t_len=T, tb=TB, tgb=TGB):
    import concourse.bass as bass  # noqa
    import concourse.mybir as mybir
    import concourse.tile as tile
    from concourse import bacc
    from concourse.masks import make_identity

    fp32 = mybir.dt.float32
    assert t_len % tgb == 0 and t_len % tb == 0

    nc = bacc.Bacc("TRN2", target_bir_lowering=False, debug=False)
    seq_d = nc.dram_tensor("seq", (t_len, V), fp32, kind="ExternalInput")
    wproj_d = nc.dram_tensor("wproj", (GL, V, K), fp32, kind="ExternalInput")
    wh_d = nc.dram_tensor("wh", (K, K), fp32, kind="ExternalInput")
    hid_d = nc.dram_tensor("hid", (128, t_len, KC, NCH), fp32, kind="ExternalOutput")
    xt_d = nc.dram_tensor("xt", (128, KC, GL, t_len), fp32)  # internal scratch

    with tile.TileContext(nc) as tc:
        # ---------------- Phase A: xprojT = W_proj^T @ seq^T ----------------
        with tc.tile_pool(name="consts", bufs=1) as cpool, \
             tc.tile_pool(name="wproj", bufs=1) as wpool, \
             tc.tile_pool(name="pa", bufs=2) as pa, \
             tc.tile_pool(name="seqT", bufs=1) as stpool, \
             tc.tile_pool(name="tps", bufs=4, space="PSUM") as tps, \
             tc.tile_pool(name="gps", bufs=2, space="PSUM") as gps:

            ident = cpool.tile([128, 128], fp32, tag="ident")
            make_identity(nc, ident[:, :])

            wp_sb = {}
            for g in range(GL):
                for vc in range(V // 128):
                    wt = wpool.tile([128, K], fp32, tag=f"wp{g}_{vc}")
                    nc.sync.dma_start(out=wt, in_=wproj_d.ap()[g, vc * 128:(vc + 1) * 128, :])
                    wp_sb[(g, vc)] = wt

            for tb0 in range(0, t_len, tgb):
                seqT = stpool.tile([128, V // 128, tgb], fp32, tag="seqT")
                for tt in range(tgb // 128):
                    seq_sb = pa.tile([128, V], fp32, tag="seq_nat")
                    nc.sync.dma_start(out=seq_sb, in_=seq_d.ap()[tb0 + tt * 128: tb0 + (tt + 1) * 128, :])
                    for vc in range(V // 128):
                        pst = tps.tile([128, 128], fp32, tag="tp")
                        nc.tensor.transpose(pst, seq_sb[:, vc * 128:(vc + 1) * 128], ident)
                        nc.vector.tensor_copy(seqT[:, vc, tt * 128:(tt + 1) * 128], pst)
                for g in range(GL):
                    for kc in range(KC):
                        ps_g = gps.tile([128, tgb], fp32, tag="gemm")
                        for vc in range(V // 128):
                            nc.tensor.matmul(
                                ps_g, wp_sb[(g, vc)][:, kc * 128:(kc + 1) * 128],
                                seqT[:, vc, :],
                                start=(vc == 0), stop=(vc == V // 128 - 1))
                        xs = pa.tile([128, tgb], fp32, tag="xstage")
                        nc.vector.tensor_copy(xs, ps_g)
                        nc.sync.dma_start(out=xt_d.ap()[:, kc, g, tb0:tb0 + tgb], in_=xs)

        # ---------------- Phase B: serial bidirectional recurrence ----------------
        with tc.tile_pool(name="whp", bufs=1) as whp, \
             tc.tile_pool(name="ob", bufs=2) as obp, \
             tc.tile_pool(name="xf", bufs=2) as xfp, \
             tc.tile_pool(name="xb", bufs=2) as xbp, \
             tc.tile_pool(name="tmp", bufs=2) as tmpp, \
             tc.tile_pool(name="psB", bufs=2, space="PSUM") as psb:

            wh_sb = whp.tile([128, KC, K], fp32, tag="wh")
            nc.sync.dma_start(
                out=wh_sb,
                in_=wh_d.ap().rearrange("(c p) n -> p c n", p=128))

            mybir_tanh = mybir.ActivationFunctionType.Tanh
            ob_prev = None
            for B in range(0, t_len, tb):
                ob_t = obp.tile([128, tb, KC, NCH], fp32, tag="ob")
                xf_t = xfp.tile([128, KC, GL, tb], fp32, tag="xf")
                nc.sync.dma_start(out=xf_t, in_=xt_d.ap()[:, :, :, B:B + tb])
                lo = max(t_len - 1 - B - tb, 0)
                xb_t = xbp.tile([128, KC, GL, tb], fp32, tag="xb")
                nc.sync.dma_start(out=xb_t, in_=xt_d.ap()[:, :, :, lo:lo + tb])

                for tl in range(tb):
                    t = B + tl
                    if t == 0:
                        nc.scalar.activation(ob_t[:, 0, :, 0:GL], xf_t[:, :, :, 0], mybir_tanh)
                        nc.scalar.activation(ob_t[:, 0, :, GL:NCH], xb_t[:, :, :, (t_len - 2) - lo], mybir_tanh)
                        continue
                    prev_tile, ptl = (ob_t, tl - 1) if tl > 0 else (ob_prev, tb - 1)
                    ps = psb.tile([128, KC, NCH], fp32, tag="ps")
                    for j in range(KC):
                        for c in range(KC):
                            nc.tensor.matmul(
                                ps[:, j, :],
                                wh_sb[:, c, j * 128:(j + 1) * 128],
                                prev_tile[:, ptl, c, :],
                                start=(c == 0), stop=(c == KC - 1))
                    tmp = tmpp.tile([128, KC, NCH], fp32, tag="tmp")
                    nc.vector.tensor_add(tmp[:, :, 0:GL], ps[:, :, 0:GL], xf_t[:, :, :, tl])
                    if t < t_len - 1:
                        bsrc = xb_t[:, :, :, (t_len - 2 - t) - lo]
                    else:
                        bsrc = xf_t[:, :, :, tb - 1]  # x_rev[T-1] = xproj[T-1]
                    nc.vector.tensor_add(tmp[:, :, GL:NCH], ps[:, :, GL:NCH], bsrc)
                    nc.scalar.activation(ob_t[:, tl, :, :], tmp, mybir_tanh)

                nc.sync.dma_start(out=hid_d.ap()[:, B:B + tb, :, :], in_=ob_t)
                ob_prev = ob_t

    nc.compile()
    return nc


def host_prep(sequence, perms, W_e, Wx, Wh, b):
    """Fold weights (input-independent) and build per-core input maps."""
    sequence = np.ascontiguousarray(np.asarray(sequence, dtype=np.float32))
    W_e = np.asarray(W_e, dtype=np.float32)
    Wx = np.asarray(Wx, dtype=np.float32)
    Wh = np.ascontiguousarray(np.asarray(Wh, dtype=np.float32))
    b = np.asarray(b, dtype=np.float32)
    perms = np.asarray(perms)

    P = W_e @ Wx + b[None, :]  # (V, K) fp32
    in_maps = []
    for core in range(NCORES):
        gs = [core * GL + i for i in range(GL)]
        wproj = np.stack([P[perms[g]] for g in gs])  # (GL, V, K): rows P[perms[g][v]]
        in_maps.append({
            "seq": sequence,
            "wproj": np.ascontiguousarray(wproj),
            "wh": Wh,
        })
    return in_maps


def postprocess(results, t_len=T):
    """results: list of per-core dicts with 'hid' (128, T, KC, NCH)."""
    hidden = np.empty((t_len, G, 2 * K), dtype=np.float32)
    for core, out in enumerate(results):
        hid = out["hid"]  # (128, t, KC, NCH)
        # h_all[t, ch, c*128+p] = hid[p, t, c, ch]
        h_all = np.transpose(hid, (1, 3, 2, 0)).reshape(t_len, NCH, K)
        hf = h_all[:, 0:GL, :]          # (t, GL, K)
        hb_raw = h_all[:, GL:NCH, :]
        hb = np.empty_like(hb_raw)
        hb[: t_len - 1] = hb_raw[: t_len - 1][::-1]
        hb[t_len - 1] = hb_raw[t_len - 1]
        gs = slice(core * GL, (core + 1) * GL)
        hidden[:, gs, 0:K] = hf
        hidden[:, gs, K:2 * K] = hb
    ht = hidden[-1].copy()
    return hidden, ht


_NC_CACHE = {}


def kernel(sequence, perms, W_e, Wx, Wh, b):
    from concourse.bass_utils import run_bass_kernel_spmd

    in_maps = host_prep(sequence, perms, W_e, Wx, Wh, b)
    key = T
    if key not in _NC_CACHE:
        _NC_CACHE[key] = build_program(T)
    nc = _NC_CACHE[key]
    res = run_bass_kernel_spmd(nc, in_maps, core_ids=list(range(NCORES)))
    return postprocess(res.results)
